# revision 3
# baseline (speedup 1.0000x reference)
"""Causal self-attention (B=4, T=2048, E=1024, H=16, D=64) on 8 TRN2 NeuronCores.

Sharding: data-parallel over batch (4) x tensor-parallel over heads (2 groups
of 8).  Core c handles batch b=c//2, head group g=c%2.

Per-core pipeline (all matmuls bf16 on TensorE, fp32 PSUM accumulation):
  A) qkv projection from pre-transposed x^T: q^T,k^T in [feat, tok] layout,
     v in natural [tok, feat] layout (with a ones column per head for the
     softmax denominator).
  B) per head: scores^T = k^T_blk.T @ q^T  ->  exp (ScalarE, scale=1/8,
     no max-subtraction: |scores|<4 for this data)  ->  causal mask multiply
     -> y^T[65, q] accumulation with v_aug (row 64 = softmax denominator Z)
     -> normalize by 1/Z broadcast via a K=1 matmul.
  C) output projection partial y^T_p[1024, 2048] -> ReduceScatter(add) over
     the neighbor pair -> + const (proj bias + proj_w @ v_bias, host-folded)
     -> out [512, 2048] fp32.

Bias algebra: k bias is softmax-shift-invariant (dropped); v bias commutes
with the (row-stochastic) attention weights so it is folded into the output
constant on the host; q bias is applied on-device.
"""

import sys

if "/opt/trn_rl_repo" not in sys.path:
    sys.path.insert(0, "/opt/trn_rl_repo")

import ml_dtypes
import numpy as np

import concourse.bass as bass
import concourse.mybir as mybir
import concourse.tile as tile
from concourse import bacc
from concourse.bass_utils import run_bass_kernel_spmd

B, T, E = 4, 2048, 1024
H, D = 16, 64
N_CORES = 8
F = 512          # local features per core (8 heads * 64)
HPC = 8          # heads per core
EC = E // 128    # 8 emb chunks
TC = T // 512    # 4 token chunks of 512
TB = T // 128    # 16 token blocks of 128
FB = F // 128    # 4 local feature blocks
OB = E // 128    # 8 output feature blocks
SCALE = 0.125    # 1/sqrt(D)

BF16 = mybir.dt.bfloat16
F32 = mybir.dt.float32
_nbf16 = ml_dtypes.bfloat16

_CACHED_NC = None


def build_nc():
    nc = bacc.Bacc("TRN2", target_bir_lowering=False, debug=False,
                   num_devices=N_CORES)

    xT = nc.declare_dram_parameter("xT", [E, T], BF16, isOutput=False)
    wqT = nc.declare_dram_parameter("wqT", [E, F], BF16, isOutput=False)
    wkT = nc.declare_dram_parameter("wkT", [E, F], BF16, isOutput=False)
    wvT = nc.declare_dram_parameter("wvT", [E, F], BF16, isOutput=False)
    pwT = nc.declare_dram_parameter("pwT", [F, E], BF16, isOutput=False)
    bqd = nc.declare_dram_parameter("bq", [128, FB], F32, isOutput=False)
    cvd = nc.declare_dram_parameter("constv", [128, FB], F32, isOutput=False)
    out = nc.declare_dram_parameter("out", [F, T], F32, isOutput=True)

    AF = mybir.ActivationFunctionType
    ALU = mybir.AluOpType

    with tile.TileContext(nc) as tc:
        with (
            tc.tile_pool(name="persist", bufs=1) as pers,
            tc.tile_pool(name="work", bufs=4) as work,
            tc.tile_pool(name="evac", bufs=3) as evac,
            tc.tile_pool(name="psA", bufs=2, space="PSUM") as psA,
            tc.tile_pool(name="psB", bufs=2, space="PSUM") as psB,
            tc.tile_pool(name="dram", bufs=1, space="DRAM") as dram,
        ):
            # ---- constants ----
            bq_t = pers.tile([128, FB], F32, tag="bq")
            cv_t = pers.tile([128, FB], F32, tag="cv")
            nc.sync.dma_start(bq_t[:], bqd[:])
            nc.sync.dma_start(cv_t[:], cvd[:])

            # causal mask bank [128, 896] bf16: [0]*384 | upper-tri(128) | [1]*384
            # window [384-j*128 : 896-j*128] masks the j-th diagonal offset.
            mask = pers.tile([128, 896], BF16, tag="mask")
            nc.gpsimd.memset(mask[:, 0:512], 0.0)
            nc.gpsimd.memset(mask[:, 512:896], 1.0)
            nc.gpsimd.affine_select(
                out=mask[:, 384:512], in_=mask[:, 384:512],
                compare_op=ALU.is_gt, fill=1.0,
                base=0, pattern=[[-1, 128]], channel_multiplier=1,
            )
            # ones row for the 1/Z broadcast matmul (K=1, M=64)
            ones64 = pers.tile([1, 64], BF16, tag="ones64")
            nc.gpsimd.memset(ones64[:], 1.0)

            # ---- persistent activations / weights ----
            xt = [pers.tile([128, T], BF16, tag=f"xT{ec}", name=f"xT{ec}") for ec in range(EC)]
            wq = [pers.tile([128, F], BF16, tag=f"wq{ec}", name=f"wq{ec}") for ec in range(EC)]
            wk = [pers.tile([128, F], BF16, tag=f"wk{ec}", name=f"wk{ec}") for ec in range(EC)]
            wv = [pers.tile([128, F], BF16, tag=f"wv{ec}", name=f"wv{ec}") for ec in range(EC)]
            pw = [pers.tile([128, E], BF16, tag=f"pw{fc}", name=f"pw{fc}") for fc in range(FB)]
            for ec in range(EC):
                sl = slice(ec * 128, (ec + 1) * 128)
                nc.sync.dma_start(xt[ec][:], xT[sl, :])
                nc.sync.dma_start(wq[ec][:], wqT[sl, :])
                nc.sync.dma_start(wk[ec][:], wkT[sl, :])
                nc.sync.dma_start(wv[ec][:], wvT[sl, :])
            for fc in range(FB):
                nc.sync.dma_start(pw[fc][:], pwT[fc * 128:(fc + 1) * 128, :])

            qT = [pers.tile([128, T], BF16, tag=f"qT{fb}", name=f"qT{fb}") for fb in range(FB)]
            kT = [pers.tile([128, T], BF16, tag=f"kT{fb}", name=f"kT{fb}") for fb in range(FB)]
            # v natural layout with per-head ones column: [vh(64) | 1] * 8
            va = [pers.tile([128, 520], BF16, tag=f"va{tb}", name=f"va{tb}") for tb in range(TB)]
            yT = [pers.tile([128, T], BF16, tag=f"yT{fb}", name=f"yT{fb}") for fb in range(FB)]

            # ---- stage A: qkv projections ----
            for fb in range(FB):
                fsl = slice(fb * 128, (fb + 1) * 128)
                for tcb in range(TC):
                    tsl = slice(tcb * 512, (tcb + 1) * 512)
                    ps = psA.tile([128, 512], F32, tag="pa")
                    for ec in range(EC):
                        nc.tensor.matmul(ps[:], wq[ec][:, fsl], xt[ec][:, tsl],
                                         start=(ec == 0), stop=(ec == EC - 1))
                    nc.vector.tensor_scalar_add(qT[fb][:, tsl], ps[:],
                                                bq_t[:, fb:fb + 1])
                    ps2 = psA.tile([128, 512], F32, tag="pa")
                    for ec in range(EC):
                        nc.tensor.matmul(ps2[:], wk[ec][:, fsl], xt[ec][:, tsl],
                                         start=(ec == 0), stop=(ec == EC - 1))
                    nc.vector.tensor_copy(kT[fb][:, tsl], ps2[:])
            for tb in range(TB):
                bsl = slice(tb * 128, (tb + 1) * 128)
                ps = psA.tile([128, 512], F32, tag="pa")
                for ec in range(EC):
                    nc.tensor.matmul(ps[:], xt[ec][:, bsl], wv[ec][:],
                                     start=(ec == 0), stop=(ec == EC - 1))
                nc.gpsimd.memset(va[tb][:], 1.0)
                for h in range(HPC):
                    nc.vector.tensor_copy(va[tb][:, h * 65:h * 65 + 64],
                                          ps[:, h * 64:(h + 1) * 64])

            # ---- stage B: attention per head ----
            for h in range(HPC):
                fb, po = h // 2, (h % 2) * 64
                qh = qT[fb][po:po + 64, :]
                kh = kT[fb][po:po + 64, :]
                for qc in range(TC):
                    qsl = slice(qc * 512, (qc + 1) * 512)
                    nkb = 4 * qc + 4
                    psy = psB.tile([65, 512], F32, tag="psy")
                    for kb in range(nkb):
                        pss = psB.tile([128, 512], F32, tag="pss")
                        nc.tensor.matmul(pss[:], kh[:, kb * 128:(kb + 1) * 128],
                                         qh[:, qsl], start=True, stop=True)
                        at = work.tile([128, 512], BF16, tag="attT")
                        nc.scalar.activation(at[:], pss[:], AF.Exp, scale=SCALE)
                        j = kb - 4 * qc
                        if j >= 0:
                            nc.vector.tensor_mul(
                                at[:], at[:],
                                mask[:, 384 - j * 128:896 - j * 128])
                        nc.tensor.matmul(psy[:], va[kb][:, h * 65:h * 65 + 65],
                                         at[:], start=(kb == 0),
                                         stop=(kb == nkb - 1))
                    # normalize: yT = psy[0:64] * (1/Z) broadcast over rows
                    rz = evac.tile([1, 512], F32, tag="rz")
                    nc.vector.reciprocal(rz[:], psy[64:65, :])
                    rzb = evac.tile([1, 512], BF16, tag="rzb")
                    nc.vector.tensor_copy(rzb[:], rz[:])
                    psb = psB.tile([64, 512], F32, tag="psb", bufs=1)
                    nc.tensor.matmul(psb[:], ones64[:], rzb[:],
                                     start=True, stop=True)
                    zb = evac.tile([64, 512], F32, tag="zb")
                    nc.vector.tensor_copy(zb[:], psb[:])
                    nc.vector.tensor_mul(yT[fb][po:po + 64, qsl],
                                         psy[0:64, :], zb[:])

            # ---- stage C: output projection + ReduceScatter ----
            yTp = dram.tile([E, T], F32)
            yTr = dram.tile([F, T], F32)
            for ob in range(OB):
                osl = slice(ob * 128, (ob + 1) * 128)
                for tcb in range(TC):
                    tsl = slice(tcb * 512, (tcb + 1) * 512)
                    ps = psA.tile([128, 512], F32, tag="pc", bufs=1)
                    for fc in range(FB):
                        nc.tensor.matmul(ps[:], pw[fc][:, osl],
                                         yT[fc][:, tsl],
                                         start=(fc == 0), stop=(fc == FB - 1))
                    st = evac.tile([128, 512], F32, tag="pjevac")
                    nc.vector.tensor_copy(st[:], ps[:])
                    nc.sync.dma_start(yTp[osl, tsl], st[:])
            nc.gpsimd.collective_compute(
                "ReduceScatter",
                ALU.add,
                replica_groups=[[0, 1], [2, 3], [4, 5], [6, 7]],
                ins=[yTp.opt()],
                outs=[yTr.opt()],
            )
            # ---- final: + const, write out ----
            for fb in range(FB):
                fsl = slice(fb * 128, (fb + 1) * 128)
                ft = work.tile([128, T], F32, tag="fin")
                nc.sync.dma_start(ft[:], yTr[fsl, :])
                nc.vector.tensor_scalar_add(ft[:], ft[:], cv_t[:, fb:fb + 1])
                nc.sync.dma_start(out[fsl, :], ft[:])

    nc.compile()
    return nc


def _get_nc():
    global _CACHED_NC
    if _CACHED_NC is None:
        _CACHED_NC = build_nc()
    return _CACHED_NC


def make_in_maps(x, qkv_w, qkv_b, proj_w, proj_b):
    x = np.asarray(x, np.float32)
    qkv_w = np.asarray(qkv_w, np.float32)
    qkv_b = np.asarray(qkv_b, np.float32)
    proj_w = np.asarray(proj_w, np.float32)
    proj_b = np.asarray(proj_b, np.float32)

    const = proj_b + proj_w @ qkv_b[2 * E:3 * E]  # v-bias folded through proj
    in_maps = []
    for c in range(N_CORES):
        b, g = c // 2, c % 2
        gsl = slice(g * F, (g + 1) * F)
        xTb = np.ascontiguousarray(x[b].T).astype(_nbf16)
        m = {
            "xT": xTb,
            "wqT": np.ascontiguousarray(qkv_w[gsl].T).astype(_nbf16),
            "wkT": np.ascontiguousarray(qkv_w[E + g * F:E + (g + 1) * F].T
                                        ).astype(_nbf16),
            "wvT": np.ascontiguousarray(qkv_w[2 * E + g * F:2 * E + (g + 1) * F].T
                                        ).astype(_nbf16),
            "pwT": np.ascontiguousarray(proj_w[:, gsl].T).astype(_nbf16),
            "bq": np.ascontiguousarray(qkv_b[gsl].reshape(FB, 128).T
                                       ).astype(np.float32),
            # ReduceScatter gives core parity p the p-th half of out feats
            "constv": np.ascontiguousarray(
                const[gsl].reshape(FB, 128).T).astype(np.float32),
        }
        in_maps.append(m)
    return in_maps


def assemble_output(results):
    y = np.empty((B, T, E), np.float32)
    for c in range(N_CORES):
        b, g = c // 2, c % 2
        y[b][:, g * F:(g + 1) * F] = results[c]["out"].T
    return y


def kernel(**inputs):
    nc = _get_nc()
    in_maps = make_in_maps(**inputs)
    res = run_bass_kernel_spmd(nc, in_maps, list(range(N_CORES)))
    return assemble_output(res.results)
